# revision 1
# baseline (speedup 1.0000x reference)
"""Trainium2 Bass kernel for LocalDualDirectedMessagePassingLayer.

Strategy (8 cores, dest-sharded):
  - Each core owns 1024 destination segments (8 blocks of 128 dests).
  - dest_seg is sorted, so each dest block's edges are contiguous; host pads
    each block's edge list to NT_B*128 and packs per core:
      srcm  [128, e_cap] bf16   node_memory[source].T
      srcf  [128, e_cap] fp8    node_features[source].T
      efts  [97, e_cap]  fp8    concat(edge_feat[edge_ids], time_enc, ones).T
      ldest [128, NT, 1] f32    per-tile local-dest column (-1 for padding)
      invc  [128, 1024]  bf16   1/cnt per dest (applied post-aggregation)
    fp8 on the feature/edge streams halves their bytes; the quantization
    noise averages out in the per-dest mean (measured ~2e-3 final err).
  - One-hot S tiles are built on the DVE: one tensor_tensor is_equal per
    4-tile sup, comparing an iota constant against ldest broadcast along
    the dest axis (stride-0 AP). No one-hot matrix is DMAed.
  - PE is software-pipelined 2 deep (read s | msg s-1 | agg s-2) so it
    never waits on the ACT relu of srT or the DVE relu of msgs.
  - Input DMA chunks (16 tiles) are spread across all three DMA queues
    (Sync/Scalar hwdge + GpSimd swdge) by a greedy balancer — the
    per-queue drain rate (~45-95GB/s) is the binding constraint.
  - Per block: dst-side MLP chain (agg/upd/write) -> tanh -> writeT.
  - Host: transpose writeT, scatter rows into a copy of node_memory.
"""

import sys

sys.path.insert(0, "/opt/trn_rl_repo")

import math

import ml_dtypes
import numpy as np

import concourse.bass as bass
import concourse.mybir as mybir
import concourse.tile as tile
from concourse import bacc
from concourse.bass_utils import run_bass_kernel_spmd

BF16 = ml_dtypes.bfloat16
FP8 = ml_dtypes.float8_e4m3
N_CORES = 8
P = 128
N_DEST = 8192
D_MEM = 128
FP8_FEAT = True

_PROG_CACHE: dict[int, object] = {}


def _chunk_plan(nt_b: int, first_block: bool):
    """DMA chunks in tiles. Chunk starts stay multiples of 4 (sup-aligned);
    chunks kept <= 12 tiles so the biggest stream (srcm bf16) stays under
    the ~512KB issue-blocking threshold of the hwdge queues."""
    if 24 < nt_b <= 40:
        chunks = [12, 12, nt_b - 24]
    else:
        tail = nt_b % 4
        body = nt_b - tail
        chunks = []
        while body > 0:
            c = min(12, body)
            chunks.append(c)
            body -= c
        if tail:
            chunks.append(tail)
    if first_block and chunks[0] >= 8:
        chunks = [4, chunks[0] - 4] + chunks[1:]
    return chunks


def _sup_plan(nt_b: int):
    sups = [4] * (nt_b // 4)
    if nt_b % 4:
        sups.append(nt_b % 4)
    return sups


def _build_program(nt_b: int):
    NT = 8 * nt_b
    e_cap = NT * P

    nc = bacc.Bacc("TRN2", target_bir_lowering=False, debug=False,
                   num_devices=N_CORES)
    f32 = mybir.dt.float32
    bf16 = mybir.dt.bfloat16
    fp8 = mybir.dt.float8e4
    featdt = fp8 if FP8_FEAT else bf16
    AF = mybir.ActivationFunctionType
    OP = mybir.AluOpType

    srcm = nc.dram_tensor("srcm", [P, e_cap], bf16, kind="ExternalInput")
    srcf = nc.dram_tensor("srcf", [P, e_cap], featdt, kind="ExternalInput")
    efts = nc.dram_tensor("efts", [97, e_cap], featdt, kind="ExternalInput")
    ldest = nc.dram_tensor("ldest", [P, NT, 1], bf16, kind="ExternalInput")
    iota = nc.dram_tensor("iota", [P, 512], bf16, kind="ExternalInput")
    big3 = nc.dram_tensor("big3", [P, 3 * 1024], bf16, kind="ExternalInput")
    crit0 = nc.dram_tensor("crit0", [P, 2 * P], bf16, kind="ExternalInput")
    wrf = nc.dram_tensor("wrf", [P, P], featdt, kind="ExternalInput")
    wm1 = nc.dram_tensor("wm1", [97, P], featdt, kind="ExternalInput")
    wsm = nc.dram_tensor("wsm", [P, 6 * P], bf16, kind="ExternalInput")
    br = nc.dram_tensor("br", [P, 1], f32, kind="ExternalInput")
    bs3 = nc.dram_tensor("bs3", [P, 3], f32, kind="ExternalInput")
    out_d = nc.dram_tensor("writeT", [P, 1024], f32, kind="ExternalOutput")

    with tile.TileContext(nc) as tc:
        with (
            tc.tile_pool(name="const", bufs=1) as cp,
            tc.tile_pool(name="io", bufs=6) as iop,
            tc.tile_pool(name="mid", bufs=8) as midp,
            tc.tile_pool(name="sp", bufs=8) as spool,
            tc.tile_pool(name="rdps", bufs=2, space="PSUM") as rdps,
            tc.tile_pool(name="mgps", bufs=3, space="PSUM") as mgps,
            tc.tile_pool(name="aggps", bufs=2, space="PSUM") as aggps,
            tc.tile_pool(name="dstps", bufs=1, space="PSUM") as dstps,
        ):
            def cload(ap, shape, dtype, tag):
                t = cp.tile(shape, dtype, tag=tag)
                nc.scalar.dma_start(out=t[:], in_=ap)
                return t

            # critical-path constants first, merged where dtypes allow
            crit0_t = cload(crit0[:, :], [P, 2 * P], bf16, "crit0")
            wr0 = crit0_t[:, 0:P]
            wm0_t = crit0_t[:, P:2 * P]
            wr1 = cload(wrf[:, :], [P, P], featdt, "wr1")
            br_t = cload(br[:, :], [P, 1], f32, "br")
            iota_t = cload(iota[:, :], [P, 4, P], bf16, "iota")
            ld_t = cload(ldest[:, :, :], [P, NT, 1], bf16, "ldest")
            wm1_t = cload(wm1[:, :], [97, P], featdt, "wm1")
            late_consts = {
                "big3": cload(big3[:, :], [P, 3 * 1024], bf16, "big3"),
                "wsm": cload(wsm[:, :], [P, 6 * P], bf16, "wsm"),
                "bs3": cload(bs3[:, :], [P, 3], f32, "bs3"),
            }

            # greedy queue balancer (sync, scalar, gpsimd), measured rates
            qload = [0.0, 14.0, 0.0]  # scalar pre-loaded with const time (us)
            qrate = [75e3, 95e3, 45e3]  # bytes/us
            qeng = [nc.sync, nc.scalar, nc.gpsimd]

            def qdma(out, in_, nbytes):
                i = min(range(3), key=lambda k: qload[k] + nbytes / qrate[k])
                qload[i] += nbytes / qrate[i]
                qeng[i].dma_start(out=out, in_=in_)

            def dst_stage(b, agg_ps, stage, hold):
                dc = slice(b * P, (b + 1) * P)
                if stage == 0:
                    mmean = midp.tile([P, P], bf16, tag="mmean")
                    nc.vector.tensor_mul(
                        mmean[:], agg_ps[:],
                        late_consts["big3"][:, 2048 + b * P:2048 + (b + 1) * P])
                    drp = dstps.tile([P, P], f32, tag="dst")
                    nc.tensor.matmul(drp[:], lhsT=wr0,
                                     rhs=late_consts["big3"][:, b * P:(b + 1) * P],
                                     start=True, stop=False)
                    nc.tensor.matmul(
                        drp[:], lhsT=late_consts["wsm"][:, 0:P],
                        rhs=late_consts["big3"][:, 1024 + b * P:1024 + (b + 1) * P],
                        start=False, stop=True)
                    dstr = midp.tile([P, P], bf16, tag="dstr")
                    nc.scalar.activation(dstr[:], drp[:], AF.Relu, bias=br_t[:, :1])
                    hold.update(mmean=mmean, dstr=dstr)
                elif stage == 1:
                    agp = dstps.tile([P, P], f32, tag="dst")
                    nc.tensor.matmul(agp[:], lhsT=late_consts["wsm"][:, P:2 * P],
                                     rhs=hold["dstr"][:],
                                     start=True, stop=False)
                    nc.tensor.matmul(agp[:], lhsT=late_consts["wsm"][:, 2 * P:3 * P],
                                     rhs=hold["mmean"][:],
                                     start=False, stop=True)
                    aggT = midp.tile([P, P], bf16, tag="aggT")
                    nc.scalar.activation(aggT[:], agp[:], AF.Relu,
                                         bias=late_consts["bs3"][:, 0:1])
                    hold.update(aggT=aggT)
                elif stage == 2:
                    upp = dstps.tile([P, P], f32, tag="dst")
                    nc.tensor.matmul(upp[:], lhsT=late_consts["wsm"][:, 3 * P:4 * P],
                                     rhs=hold["aggT"][:],
                                     start=True, stop=False)
                    nc.tensor.matmul(upp[:], lhsT=late_consts["wsm"][:, 4 * P:5 * P],
                                     rhs=hold["dstr"][:],
                                     start=False, stop=True)
                    updT = midp.tile([P, P], bf16, tag="updT")
                    nc.scalar.activation(updT[:], upp[:], AF.Relu,
                                         bias=late_consts["bs3"][:, 1:2])
                    hold.update(updT=updT)
                else:
                    wrp = dstps.tile([P, P], f32, tag="dst")
                    nc.tensor.matmul(wrp[:], lhsT=late_consts["wsm"][:, 5 * P:6 * P],
                                     rhs=hold["updT"][:],
                                     start=True, stop=True)
                    wout = midp.tile([P, P], f32, tag="wout")
                    nc.scalar.activation(wout[:], wrp[:], AF.Tanh,
                                         bias=late_consts["bs3"][:, 2:3])
                    nc.sync.dma_start(out=out_d[:, dc], in_=wout[:])

            sups = _sup_plan(nt_b)
            csz_max = max(max(_chunk_plan(nt_b, fb)) for fb in (True, False))

            sup_list = []
            for b in range(8):
                chunks = _chunk_plan(nt_b, b == 0)
                coff, chunk_bounds = 0, []
                for csz in chunks:
                    chunk_bounds.append((coff, csz))
                    coff += csz
                soff, ci = 0, 0
                for ntile in sups:
                    newc = None
                    if ci < len(chunk_bounds) and chunk_bounds[ci][0] == soff:
                        newc = chunk_bounds[ci]
                        ci += 1
                    sup_list.append((b, b * nt_b + soff, ntile, newc))
                    soff += ntile

            pending = None       # (block, agg_ps) awaiting dst stages
            hold = {}
            dst_ct = 4
            agg_ps = aggps.tile([P, P], f32, tag="agg")

            def emit_agg(pv):
                nonlocal pending, agg_ps, dst_ct, hold
                (pb, ptile0, pnt, pmsgs, pS4) = pv
                t_in_b = ptile0 - pb * nt_b
                for q in range(pnt):
                    qs = slice(q * P, (q + 1) * P)
                    nc.tensor.matmul(agg_ps[:], lhsT=pmsgs[:, qs],
                                     rhs=pS4[:, q, :],
                                     start=(t_in_b + q == 0),
                                     stop=(t_in_b + q == nt_b - 1))
                if t_in_b + pnt == nt_b:
                    pending = (pb, agg_ps)
                    dst_ct = 0
                    hold = {}
                    agg_ps = aggps.tile([P, P], f32, tag="agg")

            def emit_msg(mv):
                """msg matmuls + S build + relu for a sup."""
                (b, tile0, ntile, srT, efs, W) = mv
                mg4 = mgps.tile([P, 512], f32, tag="mg")
                for q in range(ntile):
                    qs = slice(q * P, (q + 1) * P)
                    nc.tensor.matmul(mg4[:, qs], lhsT=srT[:, qs], rhs=wm0_t,
                                     start=True, stop=False)
                    nc.tensor.matmul(mg4[:, qs], lhsT=efs[:, qs], rhs=wm1_t[:],
                                     start=False, stop=True)
                t0 = tile0
                S4 = spool.tile([P, 4, P], bf16, tag="S4")
                ld_b = ld_t[:, t0:t0 + ntile, :].to_broadcast([P, ntile, P])
                nc.vector.tensor_tensor(S4[:, :ntile, :],
                                        iota_t[:, :ntile, :],
                                        ld_b, OP.is_equal)
                msgs4 = midp.tile([P, 512], bf16, tag="msgs")
                nc.vector.tensor_scalar_max(msgs4[:, :W], mg4[:, :W], 0.0)
                return (b, tile0, ntile, msgs4, S4)

            cur_chunk = None
            pend_msg = None
            pend_agg = None
            for s, (b, tile0, ntile, newc) in enumerate(sup_list):
                if newc is not None:
                    c_t0, c_sz = newc
                    w = c_sz * P
                    o = (b * nt_b + c_t0) * P
                    cm = iop.tile([P, csz_max * P], bf16, tag="srcm")
                    qdma(cm[:, :w], srcm[:, o:o + w], 2 * w * P)
                    cf = iop.tile([P, csz_max * P], featdt, tag="srcf")
                    qdma(cf[:, :w], srcf[:, o:o + w],
                         (1 if FP8_FEAT else 2) * w * P)
                    ef = iop.tile([97, csz_max * P], featdt, tag="ef")
                    qdma(ef[:, :w], efts[:, o:o + w],
                         (1 if FP8_FEAT else 2) * w * 97)
                    cur_chunk = (cm, cf, ef, w, c_t0)

                cm, cf, ef, w, c_t0 = cur_chunk
                j = (tile0 - b * nt_b - c_t0) * P
                W = ntile * P

                # read MLP for sup s
                rd = rdps.tile([P, 512], f32, tag="rd")
                nc.tensor.matmul(rd[:, :W], lhsT=wr0, rhs=cm[:, j:j + W],
                                 start=True, stop=False)
                nc.tensor.matmul(rd[:, :W], lhsT=wr1[:], rhs=cf[:, j:j + W],
                                 start=False, stop=True)
                srT = midp.tile([P, 512], bf16, tag="srT")
                nc.scalar.activation(srT[:, :W], rd[:, :W], AF.Relu,
                                     bias=br_t[:, :1])

                # pipeline: msg for s-1, agg for s-2
                new_agg = emit_msg(pend_msg) if pend_msg is not None else None
                if pend_agg is not None:
                    emit_agg(pend_agg)
                pend_agg = new_agg
                pend_msg = (b, tile0, ntile, srT, ef[:, j:j + W], W)

                if pending is not None and dst_ct < 4:
                    dst_stage(pending[0], pending[1], dst_ct, hold)
                    dst_ct += 1

            # drain pipeline
            new_agg = emit_msg(pend_msg)
            if pend_agg is not None:
                emit_agg(pend_agg)
            emit_agg(new_agg)
            while dst_ct < 4:
                dst_stage(pending[0], pending[1], dst_ct, hold)
                dst_ct += 1

    nc.finalize()
    return nc


def _prep_inputs(inputs):
    """Host-side shard/pack. Returns (in_maps, nt_b, node_memory, node_ids)."""
    node_memory = np.ascontiguousarray(np.asarray(inputs["node_memory"], np.float32))
    node_features = np.asarray(inputs["node_features"], np.float32)
    edge_features = np.asarray(inputs["edge_features"], np.float32)
    time_encoding = np.asarray(inputs["time_encoding"], np.float32)
    node_ids = np.asarray(inputs["node_ids"]).astype(np.int64)
    source_ids = np.asarray(inputs["source_ids"]).astype(np.int64)
    edge_ids = np.asarray(inputs["edge_ids"]).astype(np.int64)
    dest_seg = np.asarray(inputs["dest_seg"]).astype(np.int64)
    W_read = np.asarray(inputs["W_read"], np.float32)
    b_read = np.asarray(inputs["b_read"], np.float32)
    W_msg = np.asarray(inputs["W_msg"], np.float32)
    b_msg = np.asarray(inputs["b_msg"], np.float32)
    W_agg = np.asarray(inputs["W_agg"], np.float32)
    b_agg = np.asarray(inputs["b_agg"], np.float32)
    W_upd = np.asarray(inputs["W_upd"], np.float32)
    b_upd = np.asarray(inputs["b_upd"], np.float32)
    W_write = np.asarray(inputs["W_write"], np.float32)
    b_write = np.asarray(inputs["b_write"], np.float32)

    n_edge = dest_seg.shape[0]

    cnt = np.bincount(dest_seg, minlength=N_DEST)
    inv_cnt = np.zeros(N_DEST, np.float32)
    nz = cnt > 0
    inv_cnt[nz] = 1.0 / cnt[nz]

    bounds = np.searchsorted(dest_seg, np.arange(0, N_DEST + 1, P))
    per_block = np.diff(bounds)
    nt_b = max(1, math.ceil(per_block.max() / P))
    block_cap = nt_b * P
    e_cap = 8 * block_cap
    NT = 8 * nt_b

    esel = np.zeros((N_CORES, e_cap), np.int64)
    valid = np.zeros((N_CORES, e_cap), bool)
    for c in range(N_CORES):
        for blk in range(8):
            B = c * 8 + blk
            lo, hi = int(bounds[B]), int(bounds[B + 1])
            off = blk * block_cap
            esel[c, off:off + hi - lo] = np.arange(lo, hi)
            valid[c, off:off + hi - lo] = True
    esel_f = esel.reshape(-1)
    valid_f = valid.reshape(-1)

    src_rows_m = node_memory[source_ids[esel_f]]           # [8*e_cap, 128]
    src_rows_f = node_features[source_ids[esel_f]]
    srcm_h = np.ascontiguousarray(
        src_rows_m.reshape(N_CORES, e_cap, P).transpose(0, 2, 1)).astype(BF16)
    srcf_h = np.ascontiguousarray(
        src_rows_f.reshape(N_CORES, e_cap, P).transpose(0, 2, 1)).astype(
            FP8 if FP8_FEAT else BF16)

    ef_rows = edge_features[edge_ids[esel_f]]
    t_rows = time_encoding[np.minimum(esel_f, n_edge - 1)]
    eft = np.concatenate(
        [ef_rows, t_rows, np.ones((len(esel_f), 1), np.float32)], axis=1)
    efts = np.ascontiguousarray(
        eft.reshape(N_CORES, e_cap, 97).transpose(0, 2, 1)).astype(
            FP8 if FP8_FEAT else BF16)

    ld_e = (dest_seg[esel_f] % P).astype(np.float32)
    ld_e[~valid_f] = -1.0
    ld_pack = np.ascontiguousarray(
        ld_e.reshape(N_CORES, NT, P).transpose(0, 2, 1))[..., None].astype(BF16)

    iota_h = np.ascontiguousarray(
        np.tile(np.arange(P, dtype=np.float32), 4)[None, :].repeat(P, 0)
    ).astype(BF16)

    invc_h = np.broadcast_to(inv_cnt.reshape(N_CORES, 1, 1024),
                             (N_CORES, P, 1024))

    nodecat = np.concatenate([node_memory, node_features], axis=1)
    drows = nodecat[node_ids]                                  # [8192, 256]
    dstT = drows.reshape(N_CORES, 1024, 256).transpose(0, 2, 1) \
        .reshape(N_CORES, 2, P, 1024)
    big3_h = np.ascontiguousarray(np.concatenate(
        [dstT[:, 0], dstT[:, 1], invc_h], axis=2)).astype(BF16)

    crit0_h = np.ascontiguousarray(
        np.concatenate([W_read[:P], W_msg[:P]], axis=1)).astype(BF16)
    wrf_h = np.ascontiguousarray(W_read[P:]).astype(FP8 if FP8_FEAT else BF16)
    wm1_h = np.ascontiguousarray(
        np.concatenate([W_msg[P:], b_msg[None, :]], axis=0)).astype(
            FP8 if FP8_FEAT else BF16)
    wsm_h = np.ascontiguousarray(np.concatenate(
        [W_read[P:], W_agg[:P], W_agg[P:], W_upd[:P], W_upd[P:], W_write],
        axis=1)).astype(BF16)
    br_h = np.ascontiguousarray(b_read[:, None]).astype(np.float32)
    bs3_h = np.ascontiguousarray(
        np.stack([b_agg, b_upd, b_write], axis=1)).astype(np.float32)

    in_maps = []
    for c in range(N_CORES):
        in_maps.append({
            "srcm": srcm_h[c], "srcf": srcf_h[c], "efts": efts[c],
            "ldest": ld_pack[c], "iota": iota_h, "big3": big3_h[c],
            "crit0": crit0_h, "wrf": wrf_h, "wm1": wm1_h,
            "wsm": wsm_h, "br": br_h, "bs3": bs3_h,
        })
    return in_maps, nt_b, node_memory, node_ids


def run(inputs, trace=False, **kw):
    in_maps, nt_b, node_memory, node_ids = _prep_inputs(inputs)
    if nt_b not in _PROG_CACHE:
        _PROG_CACHE[nt_b] = _build_program(nt_b)
    nc = _PROG_CACHE[nt_b]
    res = run_bass_kernel_spmd(nc, in_maps, core_ids=list(range(N_CORES)),
                               trace=trace, **kw)
    wt = np.concatenate(
        [np.asarray(res.results[c]["writeT"], np.float32).T
         for c in range(N_CORES)], axis=0)             # [8192, 128]
    out = node_memory.copy()
    out[node_ids] = wt
    return out, res


def kernel(**inputs) -> np.ndarray:
    out, _ = run(inputs, trace=False)
    return out

